# revision 27
# baseline (speedup 1.0000x reference)
"""Trainium2 Bass kernel for nn_DynamicsLookAheadModel.

LSTM warm-up over S=96 steps + 32-step look-ahead with output feedback,
data-parallel over the batch (2048) across 8 NeuronCores (256 per core).

Per-core layout (all fp32):
  - Everything "transposed": hidden units on partitions, batch on the free dim.
    H=256 tensors are folded: gate-dim halves 0:128 / 128:256 live in separate
    M-tiles (m = 2*gate + half).
  - Gates g = W_ih@x + W_hh@h + b computed on the PE into PSUM.
  - x-projection (K=32) runs as ONE K=128 matmul per M-tile with the four
    bf16 hi/lo product terms stacked along K:
      wst rows = [Whi; Wlo; Whi; Wlo[0:30]; bias_hi; bias_lo]
      xst rows = [xhi; xhi; xlo; xlo[0:30]; 1.0;  1.0]
    Full hi/lo precision costs 1 stream pass instead of 3 (matmul time scales
    with N only; K<=128 is free), and the bias rides along on two constant-1
    rows so the ScalarE bias port is not needed -> gate activations can be
    merged into [128,512] instructions spanning both gate-dim halves.
    (Features 30,31 lose only the Wlo@xlo second-order term, ~2^-18.)
  - h part (K=256 -> 2 K-tiles, already K-saturated): split bf16 hi+lo,
    3 exact products (Whi@hhi + Wlo@hhi + Whi@hlo) ~ fp32 to ~1e-5.
  - PSUM: one [128,512] bank per GATE (m-even | m-odd column halves).
    PSUM accumulation state is BANK-granular: a second start=True into the
    other half of a bank kills the first half's open group, so the two
    column-half groups of each bank run SEQUENTIALLY (open even, accumulate,
    close; then odd). The 4 even-half x matmuls are hoisted to the front of
    each step so the PE has independent work while the previous step's
    elementwise tail finishes.
  - ScalarE: sigmoid/tanh per M-tile [128,256] (no bias port needed) + tanh(c).
  - VectorE: cell update fp32; h = sigma(o)*tanh(c) with bf16 hi/lo split.
  - STE binarization: bits = (c' > 0) (sigmoid(o)>0, tanh sign-preserving),
    one merged is_gt -> fp16.
  - Output projection: fp16 hi/lo W_fc (4 accumulating matmuls), bits exact in
    fp16. The po accumulator aliases the current step's o-gate bank (cols 0:256
    of partitions 0:6), whose gate values are already consumed by then.
  - LA feedback: o split into bf16 hi/lo on-device, written into the stacked
    x tile rows (0:6, 32:38 hi; 64:70, 96:102 lo); the LA x matmuls issue
    AFTER the h matmuls (closing the accumulation) so the feedback wait hides
    under the h part.
  - Outputs stored per step as [6, 256], assembled as [33, 6, 256] in DRAM;
    the host gather transposes to [256, 33, 6].
"""

import numpy as np

import concourse.bass as bass
import concourse.mybir as mybir
import concourse.tile as tile
from concourse.bass_utils import run_bass_kernel_spmd

B, S, F, H, O = 2048, 96, 32, 256, 6
LA = 32
NCORES = 8
BL = B // NCORES  # 256 per-core batch
FD = 2 * BL  # 512: merged free dim (both gate-dim halves)
FP32 = mybir.dt.float32
BF16 = mybir.dt.bfloat16
FP16 = mybir.dt.float16


# --- workaround: this walrus build allows only ONE sem wait per instruction ---
def _spill_excess_waits(nc, limit=1):
    cnt = 0
    for f in nc.m.functions:
        for bb in f.blocks:
            new_list = []
            for ins in bb.instructions:
                si = ins.sync_info
                if si and si.on_wait and len(si.on_wait) > limit:
                    waits = list(si.on_wait)
                    for w in waits[:-limit]:
                        n = mybir.InstNoOp(name=f"wspill_{cnt}", ins=[], outs=[])
                        cnt += 1
                        n.engine = ins.engine
                        n.sync_info = mybir.SyncInfo(on_wait=[w], on_update=[])
                        new_list.append(n)
                    ins.sync_info = mybir.SyncInfo(
                        on_wait=waits[-limit:], on_update=list(si.on_update)
                    )
                new_list.append(ins)
            bb.instructions[:] = new_list
    return cnt


def build_nc(n_warm=S, n_la=LA, spill=True):
    from contextlib import ExitStack

    nc = bass.Bass()
    AF = mybir.ActivationFunctionType
    ALU = mybir.AluOpType

    xst_d = nc.dram_tensor("xst", [n_warm, 128, BL], BF16, kind="ExternalInput")
    xla_d = nc.dram_tensor("xla", [n_la, 128, BL], BF16, kind="ExternalInput")
    wst_d = nc.dram_tensor("wst", [128, 4 * H], BF16, kind="ExternalInput")
    whh0h_d = nc.dram_tensor("whh0h", [128, 4 * H], BF16, kind="ExternalInput")
    whh0l_d = nc.dram_tensor("whh0l", [128, 4 * H], BF16, kind="ExternalInput")
    whh1h_d = nc.dram_tensor("whh1h", [128, 4 * H], BF16, kind="ExternalInput")
    whh1l_d = nc.dram_tensor("whh1l", [128, 4 * H], BF16, kind="ExternalInput")
    wfch_d = nc.dram_tensor("wfch", [128, 2 * O], FP16, kind="ExternalInput")
    wfcl_d = nc.dram_tensor("wfcl", [128, 2 * O], FP16, kind="ExternalInput")
    bfc_d = nc.dram_tensor("bfc", [O, 1], FP32, kind="ExternalInput")
    out_d = nc.dram_tensor("out_t", [n_la + 1, O, BL], FP32, kind="ExternalOutput")

    with tile.TileContext(nc) as tc, ExitStack() as es:
        wp_ctx = es.enter_context(tc.tile_pool(name="weights", bufs=1))
        xp_ctx = es.enter_context(tc.tile_pool(name="xtiles", bufs=1))
        sp_ctx = es.enter_context(tc.tile_pool(name="state", bufs=2))
        gp_ctx = es.enter_context(tc.tile_pool(name="gates", bufs=1, space="PSUM"))

        wst = wp_ctx.tile([128, 4 * H], BF16, tag="wst")
        nc.sync.dma_start(out=wst, in_=wst_d[:, :])
        whh0h = wp_ctx.tile([128, 4 * H], BF16, tag="whh0h")
        nc.sync.dma_start(out=whh0h, in_=whh0h_d[:, :])
        whh0l = wp_ctx.tile([128, 4 * H], BF16, tag="whh0l")
        nc.sync.dma_start(out=whh0l, in_=whh0l_d[:, :])
        whh1h = wp_ctx.tile([128, 4 * H], BF16, tag="whh1h")
        nc.sync.dma_start(out=whh1h, in_=whh1h_d[:, :])
        whh1l = wp_ctx.tile([128, 4 * H], BF16, tag="whh1l")
        nc.sync.dma_start(out=whh1l, in_=whh1l_d[:, :])
        whh = [(whh0h, whh0l), (whh1h, whh1l)]
        wfch = wp_ctx.tile([128, 2 * O], FP16, tag="wfch")
        nc.sync.dma_start(out=wfch, in_=wfch_d[:, :])
        wfcl = wp_ctx.tile([128, 2 * O], FP16, tag="wfcl")
        nc.sync.dma_start(out=wfcl, in_=wfcl_d[:, :])
        bfc = wp_ctx.tile([O, 1], FP32, tag="bfc")
        nc.sync.dma_start(out=bfc, in_=bfc_d[:, :])

        xpt = []
        for t in range(n_warm):
            xt = xp_ctx.tile([128, BL], BF16, tag=f"xs{t}")
            nc.sync.dma_start(out=xt, in_=xst_d[t, :, :])
            xpt.append(xt)
        xlat = []
        for k in range(n_la):
            xtile = xp_ctx.tile([128, BL], BF16, tag=f"xla{k}")
            nc.sync.dma_start(out=xtile, in_=xla_d[k, :, :])
            xlat.append(xtile)

        GATE_FUNC = [AF.Sigmoid, AF.Sigmoid, AF.Tanh, AF.Sigmoid]  # i, f, g, o
        # bank close order for the h part: g first (c-chain is longest), o last
        EMIT_ORDER = [2, 1, 0, 3]

        def alloc_banks():
            # bank per gate: [128, 512] = (m-even cols 0:256 | m-odd 256:512)
            return [
                gp_ctx.tile([128, FD], FP32, name=f"gb{g}", tag=f"gb{g}")
                for g in range(4)
            ]

        def psl(banks, m):
            g, half = m // 2, m % 2
            return banks[g][:, half * BL : half * BL + BL]

        def x_mm(banks, m, xslice, first=False, x_last=False):
            nc.tensor.matmul(
                psl(banks, m),
                wst[:, 128 * m : 128 * m + 128],
                xslice,
                start=not x_last,
                stop=first or x_last,
                skip_group_check=True,
            )

        def h_mms(banks, m, h_prev, x_last=False):
            # one M-tile's h part: 2 K-tiles x 3 bf16 products
            col = 128 * m
            for k in (0, 1):
                wh, wl = whh[k]
                hhi, hlo = h_prev[k]
                prods = [(wh, hhi), (wl, hhi), (wh, hlo)]
                for j, (w_t, h_t) in enumerate(prods):
                    last = k == 1 and j == 2
                    nc.tensor.matmul(
                        psl(banks, m),
                        w_t[:, col : col + 128],
                        h_t[:, :],
                        start=(x_last and k == 0 and j == 0),
                        stop=(last and not x_last),
                        skip_group_check=True,
                    )

        def lstm_tail(banks, xtile, h_prev, c_prev, first=False,
                      x_last=False, want_bits=False):
            # PSUM accumulation state is BANK-granular: only one open group
            # per bank at a time. Per gate bank the two column-half groups run
            # sequentially: open even half, accumulate, close; then odd half.
            xs = xtile[:, :]
            if first:
                # f gate is unused at step 0 (c=0) but its bank is still
                # written: every bank then has a uniform alloc/write/release
                # pattern per step, so the tile pool's cross-step WAR joins
                # don't fall back to the under-synced min-join path.
                for g in EMIT_ORDER:
                    x_mm(banks, 2 * g, xs, first=True)
                    x_mm(banks, 2 * g + 1, xs, first=True)
            elif x_last:
                # LA: h part opens, the o-dependent x matmul closes (its wait
                # on the fed-back output hides under the h part)
                for g in EMIT_ORDER:
                    for m in (2 * g, 2 * g + 1):
                        h_mms(banks, m, h_prev, x_last=True)
                        x_mm(banks, m, xs, x_last=True)
            else:
                # hoist the even-half x matmuls (4 bank opens) so the PE has
                # x work while the previous step's tail finishes
                for g in EMIT_ORDER:
                    x_mm(banks, 2 * g, xs)
                for g in EMIT_ORDER:
                    h_mms(banks, 2 * g, h_prev)
                    x_mm(banks, 2 * g + 1, xs)
                    h_mms(banks, 2 * g + 1, h_prev)


            # activations: one instr per M-tile (per-half, whole-tile outputs)
            act = {}
            for g in EMIT_ORDER:
                if first and g == 1:
                    continue
                for half in (0, 1):
                    ah = sp_ctx.tile([128, BL], FP32, tag=f"a{g}_{half}")
                    act[(g, half)] = ah
                    nc.scalar.activation(
                        out=ah, in_=psl(banks, 2 * g + half), func=GATE_FUNC[g]
                    )

            c_new = []
            h_new = []
            bits_new = []
            for half in (0, 1):
                cn = sp_ctx.tile([128, BL], FP32, tag=f"c{half}")
                if first:
                    nc.vector.tensor_tensor(
                        out=cn, in0=act[(0, half)], in1=act[(2, half)], op=ALU.mult
                    )
                else:
                    t1 = sp_ctx.tile([128, BL], FP32, tag=f"t1_{half}")
                    nc.vector.tensor_tensor(
                        out=t1, in0=act[(1, half)], in1=c_prev[half], op=ALU.mult
                    )
                    t2 = sp_ctx.tile([128, BL], FP32, tag=f"t2_{half}")
                    nc.vector.tensor_tensor(
                        out=t2, in0=act[(0, half)], in1=act[(2, half)], op=ALU.mult
                    )
                    nc.vector.tensor_tensor(out=cn, in0=t1, in1=t2, op=ALU.add)
                c_new.append(cn)
                if want_bits:
                    bt = sp_ctx.tile([128, BL], FP16, tag=f"bits{half}")
                    nc.vector.tensor_scalar(
                        out=bt, in0=cn, scalar1=0.0, scalar2=None, op0=ALU.is_gt
                    )
                    bits_new.append(bt)
                tc_h = sp_ctx.tile([128, BL], FP32, tag=f"tc{half}")
                nc.scalar.activation(out=tc_h, in_=cn, func=AF.Tanh)
                hhi = sp_ctx.tile([128, BL], BF16, tag=f"hhi{half}")
                nc.vector.tensor_tensor(
                    out=hhi, in0=act[(3, half)], in1=tc_h, op=ALU.mult
                )
                hn = sp_ctx.tile([128, BL], FP32, tag=f"h{half}")
                nc.vector.tensor_tensor(
                    out=hn, in0=act[(3, half)], in1=tc_h, op=ALU.mult
                )
                hlo = sp_ctx.tile([128, BL], BF16, tag=f"hlo{half}")
                nc.vector.scalar_tensor_tensor(
                    out=hlo, in0=hhi, scalar=-1.0, in1=hn,
                    op0=ALU.mult, op1=ALU.add,
                )
                h_new.append((hhi, hlo))
            return h_new, c_new, bits_new

        def emit_output(k, banks_prev, bits_cur):
            # po aliases the already-consumed o-gate bank of the prev step
            po = banks_prev[3][0:O, 0:BL]
            pieces = [(0, wfch), (0, wfcl), (1, wfch), (1, wfcl)]
            for i, (half, wt) in enumerate(pieces):
                nc.tensor.matmul(
                    po,
                    wt[:, O * half : O * half + O],
                    bits_cur[half][:, :],
                    start=(i == 0),
                    stop=(i == len(pieces) - 1),
                    skip_group_check=True,
                )
            osb = sp_ctx.tile([O, BL], FP32, tag="osb")
            nc.scalar.activation(out=osb, in_=po, func=AF.Identity, bias=bfc)
            nc.sync.dma_start(out=out_d[k, :, :], in_=osb)
            return osb

        # warmup
        h_prev = None
        c_prev = None
        bits = None
        banks = None
        for t in range(n_warm):
            last = t == n_warm - 1
            new_banks = alloc_banks()
            h_prev, c_prev, bits_n = lstm_tail(
                new_banks, xpt[t], h_prev, c_prev, first=(t == 0), want_bits=last
            )
            if last:
                bits = bits_n
            banks = new_banks

        # look-ahead
        for k in range(n_la + 1):
            osb = emit_output(k, banks, bits)
            if k < n_la:
                xv = xlat[k]
                obf = sp_ctx.tile([O, BL], BF16, tag="obf")
                nc.vector.tensor_copy(out=obf, in_=osb)
                olo = sp_ctx.tile([O, BL], BF16, tag="olo")
                nc.vector.scalar_tensor_tensor(
                    out=olo, in0=obf, scalar=-1.0, in1=osb,
                    op0=ALU.mult, op1=ALU.add,
                )
                nc.vector.tensor_copy(out=xv[0:O, :], in_=obf)
                nc.scalar.copy(out=xv[32 : 32 + O, :], in_=obf)
                nc.vector.tensor_copy(out=xv[64 : 64 + O, :], in_=olo)
                nc.scalar.copy(out=xv[96 : 96 + O, :], in_=olo)
                new_banks = alloc_banks()
                h_prev, c_prev, bits = lstm_tail(
                    new_banks, xv, h_prev, c_prev, x_last=True, want_bits=True
                )
                banks = new_banks

    if spill:
        _spill_excess_waits(nc)
    return nc


def _stack_x(xT):
    """xT: [T, F, N] fp32 -> K-stacked [T, 128, N] bf16 with bias-ones rows."""
    import ml_dtypes as mld

    T, Fdim, N = xT.shape
    x_hi = xT.astype(mld.bfloat16)
    x_lo = (xT - x_hi.astype(np.float32)).astype(mld.bfloat16)
    out = np.zeros((T, 128, N), dtype=mld.bfloat16)
    out[:, 0:32] = x_hi
    out[:, 32:64] = x_hi
    out[:, 64:96] = x_lo
    out[:, 96:126] = x_lo[:, 0:30]
    out[:, 126:128] = np.ones((T, 2, N), dtype=mld.bfloat16)
    return out


def _host_prep(x, W_ih, W_hh, b_ih, b_hh, W_fc, b_fc):
    """Build the 8 per-core input maps."""
    x = np.asarray(x, dtype=np.float32)
    W_ih = np.asarray(W_ih, dtype=np.float32)
    W_hh = np.asarray(W_hh, dtype=np.float32)
    b_ih = np.asarray(b_ih, dtype=np.float32)
    b_hh = np.asarray(b_hh, dtype=np.float32)
    W_fc = np.asarray(W_fc, dtype=np.float32)
    b_fc = np.asarray(b_fc, dtype=np.float32)
    import ml_dtypes as mld

    bias = (b_ih + b_hh).astype(np.float32)  # [1024]
    bias_hi = bias.astype(mld.bfloat16)
    bias_lo = (bias - bias_hi.astype(np.float32)).astype(mld.bfloat16)
    w32t = np.ascontiguousarray(W_ih.T).astype(np.float32)  # [32, 1024]
    w_hi = w32t.astype(mld.bfloat16)
    w_lo = (w32t - w_hi.astype(np.float32)).astype(mld.bfloat16)
    wst = np.zeros((128, 4 * H), dtype=mld.bfloat16)
    wst[0:32] = w_hi
    wst[32:64] = w_lo
    wst[64:96] = w_hi
    wst[96:126] = w_lo[0:30]
    wst[126] = bias_hi
    wst[127] = bias_lo

    whh_t = np.ascontiguousarray(W_hh.T).astype(np.float32)  # [256, 1024]
    whh_hi = whh_t.astype(mld.bfloat16)
    whh_lo = (whh_t - whh_hi.astype(np.float32)).astype(mld.bfloat16)
    wfc_fold = np.concatenate([W_fc.T[:128], W_fc.T[128:]], axis=1)  # [128, 12]
    wfc_hi = wfc_fold.astype(np.float16)
    wfc_lo = (wfc_fold - wfc_hi.astype(np.float32)).astype(np.float16)
    shared = {
        "wst": np.ascontiguousarray(wst),
        "whh0h": np.ascontiguousarray(whh_hi[:128]),
        "whh0l": np.ascontiguousarray(whh_lo[:128]),
        "whh1h": np.ascontiguousarray(whh_hi[128:]),
        "whh1l": np.ascontiguousarray(whh_lo[128:]),
        "wfch": np.ascontiguousarray(wfc_hi),
        "wfcl": np.ascontiguousarray(wfc_lo),
        "bfc": np.ascontiguousarray(b_fc.reshape(O, 1)).astype(np.float32),
    }
    in_maps = []
    for c in range(NCORES):
        xc = x[c * BL : (c + 1) * BL]  # [BL, S, F]
        xT = np.ascontiguousarray(xc.transpose(1, 2, 0)).astype(np.float32)  # [S,F,BL]
        in_maps.append(
            {
                "xst": np.ascontiguousarray(_stack_x(xT)),
                "xla": np.ascontiguousarray(_stack_x(xT[:LA])),
                **shared,
            }
        )
    return in_maps


_NC_CACHE = {}


def _get_nc():
    if "nc" not in _NC_CACHE:
        _NC_CACHE["nc"] = build_nc()
    return _NC_CACHE["nc"]


def run(inputs, trace=False):
    in_maps = _host_prep(**inputs)
    nc = _get_nc()
    res = run_bass_kernel_spmd(nc, in_maps, core_ids=list(range(NCORES)), trace=trace)
    outs = []
    for c in range(NCORES):
        o = res.results[c]["out_t"]  # [33, 6, BL]
        outs.append(np.ascontiguousarray(o.transpose(2, 0, 1)))  # [BL, 33, 6]
    full = np.concatenate(outs, axis=0).astype(np.float32)  # [B, 33, 6]
    return full, res


def kernel(**inputs):
    full, _ = run(inputs, trace=False)
    return full


if __name__ == "__main__":
    t = build_nc()
    print("built ok")


# revision 28
# speedup vs baseline: 1.0012x; 1.0012x over previous
"""Trainium2 Bass kernel for nn_DynamicsLookAheadModel.

LSTM warm-up over S=96 steps + 32-step look-ahead with output feedback,
data-parallel over the batch (2048) across 8 NeuronCores (256 per core).

Per-core layout (all fp32):
  - Everything "transposed": hidden units on partitions, batch on the free dim.
    H=256 tensors are folded: gate-dim halves 0:128 / 128:256 live in separate
    M-tiles (m = 2*gate + half).
  - Gates g = W_ih@x + W_hh@h + b computed on the PE into PSUM.
  - x-projection (K=32) runs as ONE K=128 matmul per M-tile with the four
    bf16 hi/lo product terms stacked along K:
      wst rows = [Whi; Wlo; Whi; Wlo[0:30]; bias_hi; bias_lo]
      xst rows = [xhi; xhi; xlo; xlo[0:30]; 1.0;  1.0]
    Full hi/lo precision costs 1 stream pass instead of 3 (matmul time scales
    with N only; K<=128 is free), and the bias rides along on two constant-1
    rows so the ScalarE bias port is not needed -> gate activations can be
    merged into [128,512] instructions spanning both gate-dim halves.
    (Features 30,31 lose only the Wlo@xlo second-order term, ~2^-18.)
  - h part (K=256 -> 2 K-tiles, already K-saturated): split bf16 hi+lo,
    3 exact products (Whi@hhi + Wlo@hhi + Whi@hlo) ~ fp32 to ~1e-5.
  - PSUM: one [128,512] bank per GATE (m-even | m-odd column halves).
    PSUM accumulation state is BANK-granular: a second start=True into the
    other half of a bank kills the first half's open group, so the two
    column-half groups of each bank run SEQUENTIALLY (open even, accumulate,
    close; then odd). The 4 even-half x matmuls are hoisted to the front of
    each step so the PE has independent work while the previous step's
    elementwise tail finishes.
  - ScalarE: sigmoid/tanh per M-tile [128,256] (no bias port needed) + tanh(c).
  - VectorE: cell update fp32; h = sigma(o)*tanh(c) with bf16 hi/lo split.
  - STE binarization: bits = (c' > 0) (sigmoid(o)>0, tanh sign-preserving),
    one merged is_gt -> fp16.
  - Output projection: fp16 hi/lo W_fc (4 accumulating matmuls), bits exact in
    fp16. The po accumulator aliases the current step's o-gate bank (cols 0:256
    of partitions 0:6), whose gate values are already consumed by then.
  - LA feedback: o split into bf16 hi/lo on-device, written into the stacked
    x tile rows (0:6, 32:38 hi; 64:70, 96:102 lo); the LA x matmuls issue
    AFTER the h matmuls (closing the accumulation) so the feedback wait hides
    under the h part.
  - Outputs stored per step as [6, 256], assembled as [33, 6, 256] in DRAM;
    the host gather transposes to [256, 33, 6].
"""

import numpy as np

import concourse.bass as bass
import concourse.mybir as mybir
import concourse.tile as tile
from concourse.bass_utils import run_bass_kernel_spmd

B, S, F, H, O = 2048, 96, 32, 256, 6
LA = 32
NCORES = 8
BL = B // NCORES  # 256 per-core batch
FD = 2 * BL  # 512: merged free dim (both gate-dim halves)
FP32 = mybir.dt.float32
BF16 = mybir.dt.bfloat16
FP16 = mybir.dt.float16


# --- workaround: this walrus build allows only ONE sem wait per instruction ---
def _spill_excess_waits(nc, limit=1):
    cnt = 0
    for f in nc.m.functions:
        for bb in f.blocks:
            new_list = []
            for ins in bb.instructions:
                si = ins.sync_info
                if si and si.on_wait and len(si.on_wait) > limit:
                    waits = list(si.on_wait)
                    for w in waits[:-limit]:
                        n = mybir.InstNoOp(name=f"wspill_{cnt}", ins=[], outs=[])
                        cnt += 1
                        n.engine = ins.engine
                        n.sync_info = mybir.SyncInfo(on_wait=[w], on_update=[])
                        new_list.append(n)
                    ins.sync_info = mybir.SyncInfo(
                        on_wait=waits[-limit:], on_update=list(si.on_update)
                    )
                new_list.append(ins)
            bb.instructions[:] = new_list
    return cnt


def build_nc(n_warm=S, n_la=LA, spill=True):
    from contextlib import ExitStack

    nc = bass.Bass()
    AF = mybir.ActivationFunctionType
    ALU = mybir.AluOpType

    xst_d = nc.dram_tensor("xst", [n_warm, 128, BL], BF16, kind="ExternalInput")
    xla_d = nc.dram_tensor("xla", [n_la, 128, BL], BF16, kind="ExternalInput")
    wst_d = nc.dram_tensor("wst", [128, 4 * H], BF16, kind="ExternalInput")
    whh0h_d = nc.dram_tensor("whh0h", [128, 4 * H], BF16, kind="ExternalInput")
    whh0l_d = nc.dram_tensor("whh0l", [128, 4 * H], BF16, kind="ExternalInput")
    whh1h_d = nc.dram_tensor("whh1h", [128, 4 * H], BF16, kind="ExternalInput")
    whh1l_d = nc.dram_tensor("whh1l", [128, 4 * H], BF16, kind="ExternalInput")
    wfch_d = nc.dram_tensor("wfch", [128, 2 * O], FP16, kind="ExternalInput")
    wfcl_d = nc.dram_tensor("wfcl", [128, 2 * O], FP16, kind="ExternalInput")
    bfc_d = nc.dram_tensor("bfc", [O, 1], FP32, kind="ExternalInput")
    out_d = nc.dram_tensor("out_t", [n_la + 1, O, BL], FP32, kind="ExternalOutput")

    with tile.TileContext(nc) as tc, ExitStack() as es:
        wp_ctx = es.enter_context(tc.tile_pool(name="weights", bufs=1))
        xp_ctx = es.enter_context(tc.tile_pool(name="xtiles", bufs=1))
        sp_ctx = es.enter_context(tc.tile_pool(name="state", bufs=2))
        gp_ctx = es.enter_context(tc.tile_pool(name="gates", bufs=2, space="PSUM"))

        wst = wp_ctx.tile([128, 4 * H], BF16, tag="wst")
        nc.sync.dma_start(out=wst, in_=wst_d[:, :])
        whh0h = wp_ctx.tile([128, 4 * H], BF16, tag="whh0h")
        nc.sync.dma_start(out=whh0h, in_=whh0h_d[:, :])
        whh0l = wp_ctx.tile([128, 4 * H], BF16, tag="whh0l")
        nc.sync.dma_start(out=whh0l, in_=whh0l_d[:, :])
        whh1h = wp_ctx.tile([128, 4 * H], BF16, tag="whh1h")
        nc.sync.dma_start(out=whh1h, in_=whh1h_d[:, :])
        whh1l = wp_ctx.tile([128, 4 * H], BF16, tag="whh1l")
        nc.sync.dma_start(out=whh1l, in_=whh1l_d[:, :])
        whh = [(whh0h, whh0l), (whh1h, whh1l)]
        wfch = wp_ctx.tile([128, 2 * O], FP16, tag="wfch")
        nc.sync.dma_start(out=wfch, in_=wfch_d[:, :])
        wfcl = wp_ctx.tile([128, 2 * O], FP16, tag="wfcl")
        nc.sync.dma_start(out=wfcl, in_=wfcl_d[:, :])
        bfc = wp_ctx.tile([O, 1], FP32, tag="bfc")
        nc.sync.dma_start(out=bfc, in_=bfc_d[:, :])

        xpt = []
        for t in range(n_warm):
            xt = xp_ctx.tile([128, BL], BF16, tag=f"xs{t}")
            nc.sync.dma_start(out=xt, in_=xst_d[t, :, :])
            xpt.append(xt)
        xlat = []
        for k in range(n_la):
            xtile = xp_ctx.tile([128, BL], BF16, tag=f"xla{k}")
            nc.sync.dma_start(out=xtile, in_=xla_d[k, :, :])
            xlat.append(xtile)

        GATE_FUNC = [AF.Sigmoid, AF.Sigmoid, AF.Tanh, AF.Sigmoid]  # i, f, g, o
        # bank close order for the h part: g first (c-chain is longest), o last
        EMIT_ORDER = [2, 1, 0, 3]

        def alloc_banks():
            # bank per gate: [128, 512] = (m-even cols 0:256 | m-odd 256:512)
            return [
                gp_ctx.tile([128, FD], FP32, name=f"gb{g}", tag=f"gb{g}")
                for g in range(4)
            ]

        def psl(banks, m):
            g, half = m // 2, m % 2
            return banks[g][:, half * BL : half * BL + BL]

        def x_mm(banks, m, xslice, first=False, x_last=False):
            nc.tensor.matmul(
                psl(banks, m),
                wst[:, 128 * m : 128 * m + 128],
                xslice,
                start=not x_last,
                stop=first or x_last,
                skip_group_check=True,
            )

        def h_mms(banks, m, h_prev, x_last=False):
            # one M-tile's h part: 2 K-tiles x 3 bf16 products
            col = 128 * m
            for k in (0, 1):
                wh, wl = whh[k]
                hhi, hlo = h_prev[k]
                prods = [(wh, hhi), (wl, hhi), (wh, hlo)]
                for j, (w_t, h_t) in enumerate(prods):
                    last = k == 1 and j == 2
                    nc.tensor.matmul(
                        psl(banks, m),
                        w_t[:, col : col + 128],
                        h_t[:, :],
                        start=(x_last and k == 0 and j == 0),
                        stop=(last and not x_last),
                        skip_group_check=True,
                    )

        def lstm_tail(banks, xtile, h_prev, c_prev, first=False,
                      x_last=False, want_bits=False):
            # PSUM accumulation state is BANK-granular: only one open group
            # per bank at a time. Per gate bank the two column-half groups run
            # sequentially: open even half, accumulate, close; then odd half.
            xs = xtile[:, :]
            if first:
                # f gate is unused at step 0 (c=0) but its bank is still
                # written: every bank then has a uniform alloc/write/release
                # pattern per step, so the tile pool's cross-step WAR joins
                # don't fall back to the under-synced min-join path.
                for g in EMIT_ORDER:
                    x_mm(banks, 2 * g, xs, first=True)
                    x_mm(banks, 2 * g + 1, xs, first=True)
            elif x_last:
                # LA: h part opens, the o-dependent x matmul closes (its wait
                # on the fed-back output hides under the h part)
                for g in EMIT_ORDER:
                    for m in (2 * g, 2 * g + 1):
                        h_mms(banks, m, h_prev, x_last=True)
                        x_mm(banks, m, xs, x_last=True)
            else:
                # hoist the even-half x matmuls (4 bank opens) so the PE has
                # x work while the previous step's tail finishes
                for g in EMIT_ORDER:
                    x_mm(banks, 2 * g, xs)
                for g in EMIT_ORDER:
                    h_mms(banks, 2 * g, h_prev)
                    x_mm(banks, 2 * g + 1, xs)
                    h_mms(banks, 2 * g + 1, h_prev)


            # activations: one instr per M-tile (per-half, whole-tile outputs)
            act = {}
            for g in EMIT_ORDER:
                if first and g == 1:
                    continue
                for half in (0, 1):
                    ah = sp_ctx.tile([128, BL], FP32, tag=f"a{g}_{half}")
                    act[(g, half)] = ah
                    nc.scalar.activation(
                        out=ah, in_=psl(banks, 2 * g + half), func=GATE_FUNC[g]
                    )

            c_new = []
            h_new = []
            bits_new = []
            for half in (0, 1):
                cn = sp_ctx.tile([128, BL], FP32, tag=f"c{half}")
                if first:
                    nc.vector.tensor_tensor(
                        out=cn, in0=act[(0, half)], in1=act[(2, half)], op=ALU.mult
                    )
                else:
                    t1 = sp_ctx.tile([128, BL], FP32, tag=f"t1_{half}")
                    nc.vector.tensor_tensor(
                        out=t1, in0=act[(1, half)], in1=c_prev[half], op=ALU.mult
                    )
                    t2 = sp_ctx.tile([128, BL], FP32, tag=f"t2_{half}")
                    nc.vector.tensor_tensor(
                        out=t2, in0=act[(0, half)], in1=act[(2, half)], op=ALU.mult
                    )
                    nc.vector.tensor_tensor(out=cn, in0=t1, in1=t2, op=ALU.add)
                c_new.append(cn)
                if want_bits:
                    bt = sp_ctx.tile([128, BL], FP16, tag=f"bits{half}")
                    nc.vector.tensor_scalar(
                        out=bt, in0=cn, scalar1=0.0, scalar2=None, op0=ALU.is_gt
                    )
                    bits_new.append(bt)
                tc_h = sp_ctx.tile([128, BL], FP32, tag=f"tc{half}")
                nc.scalar.activation(out=tc_h, in_=cn, func=AF.Tanh)
                hhi = sp_ctx.tile([128, BL], BF16, tag=f"hhi{half}")
                nc.vector.tensor_tensor(
                    out=hhi, in0=act[(3, half)], in1=tc_h, op=ALU.mult
                )
                hn = sp_ctx.tile([128, BL], FP32, tag=f"h{half}")
                nc.vector.tensor_tensor(
                    out=hn, in0=act[(3, half)], in1=tc_h, op=ALU.mult
                )
                hlo = sp_ctx.tile([128, BL], BF16, tag=f"hlo{half}")
                nc.vector.scalar_tensor_tensor(
                    out=hlo, in0=hhi, scalar=-1.0, in1=hn,
                    op0=ALU.mult, op1=ALU.add,
                )
                h_new.append((hhi, hlo))
            return h_new, c_new, bits_new

        def emit_output(k, banks_prev, bits_cur):
            # po aliases the already-consumed o-gate bank of the prev step
            po = banks_prev[3][0:O, 0:BL]
            pieces = [(0, wfch), (0, wfcl), (1, wfch), (1, wfcl)]
            for i, (half, wt) in enumerate(pieces):
                nc.tensor.matmul(
                    po,
                    wt[:, O * half : O * half + O],
                    bits_cur[half][:, :],
                    start=(i == 0),
                    stop=(i == len(pieces) - 1),
                    skip_group_check=True,
                )
            osb = sp_ctx.tile([O, BL], FP32, tag="osb")
            nc.scalar.activation(out=osb, in_=po, func=AF.Identity, bias=bfc)
            nc.sync.dma_start(out=out_d[k, :, :], in_=osb)
            return osb

        # warmup
        h_prev = None
        c_prev = None
        bits = None
        banks = None
        for t in range(n_warm):
            last = t == n_warm - 1
            new_banks = alloc_banks()
            h_prev, c_prev, bits_n = lstm_tail(
                new_banks, xpt[t], h_prev, c_prev, first=(t == 0), want_bits=last
            )
            if last:
                bits = bits_n
            banks = new_banks

        # look-ahead
        for k in range(n_la + 1):
            osb = emit_output(k, banks, bits)
            if k < n_la:
                xv = xlat[k]
                obf = sp_ctx.tile([O, BL], BF16, tag="obf")
                nc.vector.tensor_copy(out=obf, in_=osb)
                olo = sp_ctx.tile([O, BL], BF16, tag="olo")
                nc.vector.scalar_tensor_tensor(
                    out=olo, in0=obf, scalar=-1.0, in1=osb,
                    op0=ALU.mult, op1=ALU.add,
                )
                nc.vector.tensor_copy(out=xv[0:O, :], in_=obf)
                nc.scalar.copy(out=xv[32 : 32 + O, :], in_=obf)
                nc.vector.tensor_copy(out=xv[64 : 64 + O, :], in_=olo)
                nc.scalar.copy(out=xv[96 : 96 + O, :], in_=olo)
                new_banks = alloc_banks()
                h_prev, c_prev, bits = lstm_tail(
                    new_banks, xv, h_prev, c_prev, x_last=True, want_bits=True
                )
                banks = new_banks

    if spill:
        _spill_excess_waits(nc)
    return nc


def _stack_x(xT):
    """xT: [T, F, N] fp32 -> K-stacked [T, 128, N] bf16 with bias-ones rows."""
    import ml_dtypes as mld

    T, Fdim, N = xT.shape
    x_hi = xT.astype(mld.bfloat16)
    x_lo = (xT - x_hi.astype(np.float32)).astype(mld.bfloat16)
    out = np.zeros((T, 128, N), dtype=mld.bfloat16)
    out[:, 0:32] = x_hi
    out[:, 32:64] = x_hi
    out[:, 64:96] = x_lo
    out[:, 96:126] = x_lo[:, 0:30]
    out[:, 126:128] = np.ones((T, 2, N), dtype=mld.bfloat16)
    return out


def _host_prep(x, W_ih, W_hh, b_ih, b_hh, W_fc, b_fc):
    """Build the 8 per-core input maps."""
    x = np.asarray(x, dtype=np.float32)
    W_ih = np.asarray(W_ih, dtype=np.float32)
    W_hh = np.asarray(W_hh, dtype=np.float32)
    b_ih = np.asarray(b_ih, dtype=np.float32)
    b_hh = np.asarray(b_hh, dtype=np.float32)
    W_fc = np.asarray(W_fc, dtype=np.float32)
    b_fc = np.asarray(b_fc, dtype=np.float32)
    import ml_dtypes as mld

    bias = (b_ih + b_hh).astype(np.float32)  # [1024]
    bias_hi = bias.astype(mld.bfloat16)
    bias_lo = (bias - bias_hi.astype(np.float32)).astype(mld.bfloat16)
    w32t = np.ascontiguousarray(W_ih.T).astype(np.float32)  # [32, 1024]
    w_hi = w32t.astype(mld.bfloat16)
    w_lo = (w32t - w_hi.astype(np.float32)).astype(mld.bfloat16)
    wst = np.zeros((128, 4 * H), dtype=mld.bfloat16)
    wst[0:32] = w_hi
    wst[32:64] = w_lo
    wst[64:96] = w_hi
    wst[96:126] = w_lo[0:30]
    wst[126] = bias_hi
    wst[127] = bias_lo

    whh_t = np.ascontiguousarray(W_hh.T).astype(np.float32)  # [256, 1024]
    whh_hi = whh_t.astype(mld.bfloat16)
    whh_lo = (whh_t - whh_hi.astype(np.float32)).astype(mld.bfloat16)
    wfc_fold = np.concatenate([W_fc.T[:128], W_fc.T[128:]], axis=1)  # [128, 12]
    wfc_hi = wfc_fold.astype(np.float16)
    wfc_lo = (wfc_fold - wfc_hi.astype(np.float32)).astype(np.float16)
    shared = {
        "wst": np.ascontiguousarray(wst),
        "whh0h": np.ascontiguousarray(whh_hi[:128]),
        "whh0l": np.ascontiguousarray(whh_lo[:128]),
        "whh1h": np.ascontiguousarray(whh_hi[128:]),
        "whh1l": np.ascontiguousarray(whh_lo[128:]),
        "wfch": np.ascontiguousarray(wfc_hi),
        "wfcl": np.ascontiguousarray(wfc_lo),
        "bfc": np.ascontiguousarray(b_fc.reshape(O, 1)).astype(np.float32),
    }
    in_maps = []
    for c in range(NCORES):
        xc = x[c * BL : (c + 1) * BL]  # [BL, S, F]
        xT = np.ascontiguousarray(xc.transpose(1, 2, 0)).astype(np.float32)  # [S,F,BL]
        in_maps.append(
            {
                "xst": np.ascontiguousarray(_stack_x(xT)),
                "xla": np.ascontiguousarray(_stack_x(xT[:LA])),
                **shared,
            }
        )
    return in_maps


_NC_CACHE = {}


def _get_nc():
    if "nc" not in _NC_CACHE:
        _NC_CACHE["nc"] = build_nc()
    return _NC_CACHE["nc"]


def run(inputs, trace=False):
    in_maps = _host_prep(**inputs)
    nc = _get_nc()
    res = run_bass_kernel_spmd(nc, in_maps, core_ids=list(range(NCORES)), trace=trace)
    outs = []
    for c in range(NCORES):
        o = res.results[c]["out_t"]  # [33, 6, BL]
        outs.append(np.ascontiguousarray(o.transpose(2, 0, 1)))  # [BL, 33, 6]
    full = np.concatenate(outs, axis=0).astype(np.float32)  # [B, 33, 6]
    return full, res


def kernel(**inputs):
    full, _ = run(inputs, trace=False)
    return full


if __name__ == "__main__":
    t = build_nc()
    print("built ok")
